# revision 5
# baseline (speedup 1.0000x reference)
"""GCN layer (gather + segment_sum + linear + relu) on 8 TRN2 NeuronCores.

Strategy (edge-cut partitioning by destination node):
  - Nodes are split into 8 contiguous ranges of 6250; core i owns all edges
    whose dst falls in its range and produces output rows [i*6250,(i+1)*6250).
  - Host sorts each core's edges by dst into 128-node windows. For each
    window the kernel gathers feature[src] rows (fp16 tables, dma_gather with
    int16 indices), builds per-tile one-hot matrices from the local dst ids
    (iota == dst via tensor_scalar on DVE) and accumulates
    h^T[f,n] += msgs^T @ onehot on the TensorEngine in PSUM.
  - Epilogue per window: h^T (PSUM) -> SBUF, out = relu(h @ W + b) via two
    matmuls (W product + rank-1 bias) and a Relu activation, DMA to DRAM.
  - The feature table is replicated per core; gather indices are int16 so the
    table is split in two row-chunks with separate gathers.
"""

import numpy as np

import concourse.bass as bass
import concourse.mybir as mybir
import concourse.tile as tile
from concourse import bacc
from concourse.bass_utils import run_bass_kernel_spmd

P = 128  # partitions / tile edge


class Cfg:
    def __init__(self, n_nodes, n_edges, d, n_cores, dt16=mybir.dt.float16):
        self.n_nodes = n_nodes
        self.n_edges = n_edges
        self.d = d
        self.n_cores = n_cores
        self.nodes_per_core = n_nodes // n_cores
        assert self.nodes_per_core * n_cores == n_nodes
        self.n_windows = (self.nodes_per_core + P - 1) // P
        self.chunk = (n_nodes + 1) // 2  # feature-table row chunks (int16 idx)
        assert self.chunk < 32768
        self.dt16 = dt16
        self.np16 = np.float16 if dt16 == mybir.dt.float16 else np.dtype("bfloat16")


CFG = Cfg(50000, 800000, 128, 8)

PAD_DST = 200.0  # one-hot miss marker (> any local dst, exact in fp16/bf16)


def _prepare(cfg, feature, edge_src, edge_dst):
    """Host-side shard/sort/pad. Returns per-core arrays + baked tile counts."""
    npc, nw = cfg.nodes_per_core, cfg.n_windows
    core = edge_dst // npc
    local = edge_dst - core * npc
    win = local >> 7
    dloc = (local & 127).astype(np.float32)
    chunk = (edge_src >= cfg.chunk).astype(np.int64)
    sloc = (edge_src - chunk * cfg.chunk).astype(np.int16)

    nkey = (core * nw + win) * 2 + chunk  # block id, blocks ordered (core,win,chunk)
    order = np.argsort(nkey, kind="stable")
    sloc, dloc, nkey = sloc[order], dloc[order], nkey[order]
    counts = np.bincount(nkey, minlength=cfg.n_cores * nw * 2).reshape(
        cfg.n_cores, nw, 2
    )
    T = (-(-counts // P)).max(axis=0)  # [nw, 2] tiles per (window, chunk), SPMD-uniform
    rows = T * P
    total_rows = int(rows.sum())
    total_tiles = int(T.sum())
    total_s = total_rows // 16

    # per-core padded edge streams
    starts = np.concatenate([[0], np.cumsum(counts.reshape(-1))])
    blk_off = np.concatenate([[0], np.cumsum(rows.reshape(-1))])  # within one core
    idx16 = np.zeros((cfg.n_cores, 128, total_s), np.int16)
    dstw = np.full((cfg.n_cores, 128, total_tiles), PAD_DST, np.float32)
    for i in range(cfg.n_cores):
        src_pad = np.zeros(total_rows, np.int16)
        dst_pad = np.full(total_rows, PAD_DST, np.float32)
        for bi in range(nw * 2):
            g = i * nw * 2 + bi
            n = counts.reshape(-1)[g]
            o = blk_off[bi]
            src_pad[o : o + n] = sloc[starts[g] : starts[g] + n]
            dst_pad[o : o + n] = dloc[starts[g] : starts[g] + n]
        # dma_gather idx layout: idx j of a gather sits at [j%16, j//16],
        # replicated across the 8 Q7 cores (16-partition groups).
        idx16[i] = np.tile(
            src_pad.reshape(total_s, 16).T, (8, 1)
        )  # blockwise: see note below
        dstw[i] = dst_pad.reshape(total_tiles, P).T
    return T, idx16, dstw, total_s, total_tiles


def _build(cfg, T):
    """Build the SPMD program. T is the baked [n_windows, 2] tile-count table."""
    f32 = mybir.dt.float32
    dt16 = cfg.dt16
    nw = cfg.n_windows
    total_tiles = int(T.sum())
    total_s = int(T.sum() * P // 16)
    is_equal = mybir.AluOpType.is_equal

    nc = bacc.Bacc(None, target_bir_lowering=False)
    feat0 = nc.declare_dram_parameter("feat0", [cfg.chunk, cfg.d], dt16, False)
    feat1 = nc.declare_dram_parameter(
        "feat1", [cfg.n_nodes - cfg.chunk, cfg.d], dt16, False
    )
    idx = nc.declare_dram_parameter("idx16", [P, total_s], mybir.dt.int16, False)
    dstw = nc.declare_dram_parameter("dstw", [P, total_tiles], f32, False)
    iota = nc.declare_dram_parameter("iota16", [P, P], dt16, False)
    wmat = nc.declare_dram_parameter("wmat", [cfg.d, cfg.d], f32, False)
    bvec = nc.declare_dram_parameter("bvec", [1, cfg.d], f32, False)
    out = nc.declare_dram_parameter("out", [cfg.nodes_per_core, cfg.d], f32, True)

    with tile.TileContext(nc) as tc:
        with (
            tc.tile_pool(name="const", bufs=1) as cpool,
            tc.tile_pool(name="msgs", bufs=3) as mpool,
            tc.tile_pool(name="oh", bufs=8) as ohpool,
            tc.tile_pool(name="ep", bufs=3) as eppool,
            tc.tile_pool(name="psA", bufs=2, space="PSUM") as psa,
            tc.tile_pool(name="psB", bufs=2, space="PSUM") as psb,
        ):
            idx_sb = cpool.tile([P, total_s], mybir.dt.int16)
            nc.sync.dma_start(idx_sb[:], idx[:])
            dst_sb = cpool.tile([P, total_tiles], f32)
            nc.sync.dma_start(dst_sb[:], dstw[:])
            iota_sb = cpool.tile([P, P], dt16)
            nc.sync.dma_start(iota_sb[:], iota[:])
            w_sb = cpool.tile([cfg.d, cfg.d], f32)
            nc.sync.dma_start(w_sb[:], wmat[:])
            b_sb = cpool.tile([1, cfg.d], f32)
            nc.sync.dma_start(b_sb[:], bvec[:])
            ones_sb = cpool.tile([1, cfg.d], f32)
            nc.vector.memset(ones_sb[:], 1.0)

            t_max = int((T[:, 0] + T[:, 1]).max())
            s_off = 0
            t_off = 0
            for j in range(nw):
                tj = int(T[j, 0] + T[j, 1])
                msgs = mpool.tile([P, t_max, cfg.d], dt16, tag="msgs")
                c_off = 0
                for c in (0, 1):
                    tc_ = int(T[j, c])
                    if tc_ == 0:
                        continue
                    # SWDGE descriptor ring holds 1024 descs; single-packet
                    # gathers must fit in it, so split into <=1024-idx calls.
                    for g0 in range(0, tc_, 8):
                        gt = min(8, tc_ - g0)
                        r = gt * P
                        nc.gpsimd.dma_gather(
                            msgs[:, c_off : c_off + gt, :],
                            (feat0 if c == 0 else feat1)[:, :],
                            idx_sb[:, s_off : s_off + r // 16],
                            r,
                            r,
                            cfg.d,
                            single_packet=True,
                        )
                        c_off += gt
                        s_off += r // 16
                psum_t = psa.tile([P, P], f32, tag="pT")
                for t in range(tj):
                    oh = ohpool.tile([P, P], dt16, tag="oh")
                    nc.vector.tensor_scalar(
                        oh[:],
                        iota_sb[:],
                        dst_sb[:, t_off + t : t_off + t + 1],
                        None,
                        is_equal,
                    )
                    nc.tensor.matmul(
                        psum_t[:],
                        lhsT=msgs[:, t, :],
                        rhs=oh[:],
                        start=(t == 0),
                        stop=(t == tj - 1),
                    )
                t_off += tj
                h_t = eppool.tile([P, P], f32, tag="hT")
                nc.vector.tensor_copy(h_t[:], psum_t[:])
                psum_o = psb.tile([P, P], f32, tag="p2")
                nc.tensor.matmul(psum_o[:], lhsT=h_t[:], rhs=w_sb[:], start=True, stop=False)
                nc.tensor.matmul(
                    psum_o[:], lhsT=ones_sb[:1, :], rhs=b_sb[:1, :], start=False, stop=True
                )
                ow = eppool.tile([P, P], f32, tag="ow")
                nc.scalar.activation(
                    ow[:], psum_o[:], mybir.ActivationFunctionType.Relu
                )
                rows = min(P, cfg.nodes_per_core - j * P)
                nc.sync.dma_start(out[j * P : j * P + rows, :], ow[:rows, :])
    nc.compile()
    return nc


def make_in_maps(cfg, feature, edge_src, edge_dst, W, b):
    feature = np.asarray(feature, np.float32)
    edge_src = np.asarray(edge_src, np.int32)
    edge_dst = np.asarray(edge_dst, np.int32)
    W = np.asarray(W, np.float32)
    b = np.asarray(b, np.float32)
    T, idx16, dstw, total_s, total_tiles = _prepare(cfg, feature, edge_src, edge_dst)
    f16 = np.ascontiguousarray(feature.astype(cfg.np16))
    feat0, feat1 = f16[: cfg.chunk], f16[cfg.chunk :]
    iota16 = np.ascontiguousarray(
        np.broadcast_to(np.arange(P, dtype=np.float32), (P, P))
    ).astype(cfg.np16)
    in_maps = [
        dict(
            feat0=feat0,
            feat1=feat1,
            idx16=np.ascontiguousarray(idx16[i]),
            dstw=np.ascontiguousarray(dstw[i]),
            iota16=iota16,
            wmat=W,
            bvec=b[None, :],
        )
        for i in range(cfg.n_cores)
    ]
    return T, in_maps


_BUILD_CACHE = {}


def run(feature, edge_src, edge_dst, W, b, cfg=CFG, trace=False, **spmd_kwargs):
    T, in_maps = make_in_maps(cfg, feature, edge_src, edge_dst, W, b)
    key = (cfg.n_nodes, cfg.n_edges, tuple(T.reshape(-1).tolist()))
    nc = _BUILD_CACHE.get(key)
    if nc is None:
        nc = _build(cfg, T)
        _BUILD_CACHE[key] = nc
    res = run_bass_kernel_spmd(
        nc, in_maps, core_ids=list(range(cfg.n_cores)), trace=trace, **spmd_kwargs
    )
    outs = [np.asarray(res.results[i]["out"]) for i in range(cfg.n_cores)]
    return np.concatenate(outs, axis=0), res


def kernel(**inputs):
    out, _ = run(
        inputs["feature"],
        inputs["edge_src"],
        inputs["edge_dst"],
        inputs["W"],
        inputs["b"],
    )
    return out
